# revision 45
# baseline (speedup 1.0000x reference)
"""AttentionEXT Trainium2 kernel: 8-core SPMD, sharded over N (ext points).

Reference computation (per point n, label m):
    A = enc1(ext_fea)  [N,256];  B = enc2(lab_fea)  [M,256]
    diff = A[n]-B[m];  wei = MLP(diff) [N,M,256]; softmax over m (per n,channel)
    att[n] = sum_m softmax(wei)*diff;  out = att @ fcw.T + fcb

Algebraic restructuring used here:
  * BN(eval) folded into weights on host: w' = g*w, b' = g*b+be.
  * MLP layer 1 is linear in diff: h1 = relu(P[n] + R[m]),
      P = A@W1'.T, R = b1' - B@W1'.T          (kills the [N*M,256]@[256,32] matmul)
  * softmax sums to 1  =>  att = A - U/Z  with
      E = exp(relu(y3)) = max(exp(y3),1), Z = sum_m E, U = sum_m E*B
    (no diff materialization, no softmax normalization pass)
All tensors on device live channel-major: [channels(partitions), tokens(free)].
Hot-loop pair columns are ordered (m outer, n inner) so the m-halving
reduction trees operate on flat contiguous slices (DVE 2x mode), and the
broadcast operands (R over n, B over n) are materialized once per core so
every hot DVE op has unit-stride innermost access.
Constants are packed into one fp32 [128, PACKF] and one bf16 [128, PACKBF]
DRAM tensor so loads are two DMAs (matmul sync-wait slots are scarce).
"""

import os
import sys

sys.path.insert(0, "/opt/trn_rl_repo")

import numpy as np
import ml_dtypes
from concourse import bass, bacc, mybir
from concourse import tile
from concourse.bass_utils import run_bass_kernel_spmd

N, M, D_IN, H1, D, OUT_C = 2048, 128, 352, 512, 256, 13
NCORES = 8
NS = N // NCORES  # 256 ext points per core
KIN = 384  # 352 padded to 3*128
NCH = 32  # points per outer chunk
F32 = mybir.dt.float32
BF = mybir.dt.bfloat16
AX = mybir.AxisListType
AF = mybir.ActivationFunctionType
ALU = mybir.AluOpType

BF_NP = ml_dtypes.bfloat16

# ---- packed fp32 constants: name -> (free words per partition) ----
_PACK_SPEC = [
    ("fcw", 2 * OUT_C),
    ("b1a", 4),
    ("b1b", 2),
    ("b2a", 4),
    ("b2b", 2),
    ("mb1", 1),        # rows 0-31
    ("mb2", 1),        # rows 0-63
    ("mb3", 2),
    ("fcb", 1),        # rows 0-12
]
_PACK_OFF = {}
_off = 0
for _nm, _w in _PACK_SPEC:
    _PACK_OFF[_nm] = _off
    _off += _w
PACKF = _off

# ---- packed bf16 constants (ext-encoder inputs first: their DMA slice
# lands first so the ext encoder starts while the rest streams in) ----
_PACKB_SPEC = [
    ("w1a", 3 * H1),   # [128,3,512]
    ("w1b", 4 * D),    # [128,4,256]
    ("mw1", 2 * 32),   # [128,2,32]
    ("xT", 3 * NS),    # per-core shard
    ("w2a", 3 * H1),
    ("w2b", 4 * D),
    ("mw2", 64),       # mw2 replicated on all 4 partition quarters
    ("mw3", D),        # rows 0-63 valid
    ("lT", 3 * M),
]
_PACKB_CUT1 = 3 * H1 + 4 * D + 2 * 32 + 3 * NS          # end of ext slice
_PACKB_CUT2 = _PACKB_CUT1 + 3 * H1 + 4 * D              # end of lab weights
_PACKB_OFF = {}
_off = 0
for _nm, _w in _PACKB_SPEC:
    _PACKB_OFF[_nm] = _off
    _off += _w
PACKBF = _off

_PROG_CACHE: dict = {}


def _build_program():
    nc = bacc.Bacc(None)
    pack_d = nc.declare_dram_parameter("pack", [128, PACKF], F32, isOutput=False)
    packb_d = nc.declare_dram_parameter("packb", [128, PACKBF], BF, isOutput=False)
    # channel-major [OUT_C, NS]; host transposes (a [NS, OUT_C] store would be
    # an element-granular scatter DMA, ~23us on one queue)
    out_d = nc.declare_dram_parameter("out", [OUT_C, NS], F32, isOutput=True)

    NCHUNKS = NS // NCH  # 8

    with tile.TileContext(nc) as tc:
        with tc.tile_pool(name="persist", bufs=1) as wp:
            pk = wp.tile([128, PACKF], F32)
            nc.sync.dma_start(pk[:], pack_d[:])
            pkb = wp.tile([128, PACKBF], BF)
            # three parallel DMA queues; lab slice arrives first
            nc.sync.dma_start(pkb[:, 0:_PACKB_CUT1], packb_d[:, 0:_PACKB_CUT1])
            nc.sync.dma_start(pkb[:, _PACKB_CUT1:_PACKB_CUT2],
                              packb_d[:, _PACKB_CUT1:_PACKB_CUT2])
            nc.sync.dma_start(pkb[:, _PACKB_CUT2:], packb_d[:, _PACKB_CUT2:])

            def sl(name, rows=128):
                a, w = _PACK_OFF[name], dict(_PACK_SPEC)[name]
                return pk[:rows, a:a + w]

            def slb(name, rows=128):
                a, w = _PACKB_OFF[name], dict(_PACKB_SPEC)[name]
                return pkb[:rows, a:a + w]

            w1a_s = slb("w1a").rearrange("p (k m) -> p k m", k=3)
            w1b_s = slb("w1b").rearrange("p (k m) -> p k m", k=4)
            w2a_s = slb("w2a").rearrange("p (k m) -> p k m", k=3)
            w2b_s = slb("w2b").rearrange("p (k m) -> p k m", k=4)
            mw1_s = slb("mw1").rearrange("p (k m) -> p k m", k=2)
            mw2b = slb("mw2")  # mw2 replicated on all 4 partition quarters
            mw3b = slb("mw3", 64)
            xT_s = slb("xT").rearrange("p (k m) -> p k m", k=3)
            lT_s = slb("lT").rearrange("p (k m) -> p k m", k=3)
            fcw_s = sl("fcw").rearrange("p (k m) -> p k m", k=2)
            b1a_s = sl("b1a")
            b1b_s = sl("b1b")
            b2a_s = sl("b2a")
            b2b_s = sl("b2b")
            mb1_s = sl("mb1", 32)
            mb2_s = sl("mb2", 64)
            mb3_s = sl("mb3")
            fcb_s = sl("fcb", OUT_C)

            # ---- encoders (bf16 matmuls, fp32 psum accumulate) ----
            B1_s = wp.tile([128, 4, M], BF)    # lab hidden [512ch, 128]
            BTb = wp.tile([128, 2, M], BF)     # B^T        [256ch, 128]
            A1_s = wp.tile([128, 4, NS], BF)   # ext hidden [512ch, 256]
            AT_s = wp.tile([128, 2, NS], F32)  # A^T        [256ch, 256]
            ATb = wp.tile([128, 2, NS], BF)
            PTb = wp.tile([32, NS], BF)
            RTb = wp.tile([32, M], BF)

            P4 = wp.tile([128, 4, NCH], BF)
            R4 = wp.tile([128, M], BF)
            R4_rep = wp.tile([128, M, NCH], BF)
            B_rep = wp.tile([128, 2, M, NCH], BF)
            h1g0 = wp.tile([128, M * NCH], BF)
            h1g1 = wp.tile([128, M * NCH], BF)
            h1g2 = wp.tile([128, M * NCH], BF)
            h1g3 = wp.tile([128, M * NCH], BF)
            h1g = [h1g0, h1g1, h1g2, h1g3]

            with tc.tile_pool(name="enc_psum", bufs=2, space="PSUM") as epp:
                # ext encoder first (convs1 weights): its outputs gate the
                # whole pairwise pipeline via P
                for mt in range(4):
                    ps = epp.tile([128, NS], F32, tag="enca")
                    for kt in range(3):
                        nc.tensor.matmul(
                            ps, w1a_s[:, kt, mt * 128:(mt + 1) * 128], xT_s[:, kt],
                            start=(kt == 0), stop=(kt == 2))
                    nc.scalar.activation(A1_s[:, mt], ps, AF.Relu,
                                         bias=b1a_s[:, mt:mt + 1])
                for mt in range(2):
                    ps = epp.tile([128, NS], F32, tag="enca")
                    for kt in range(4):
                        nc.tensor.matmul(
                            ps, w1b_s[:, kt, mt * 128:(mt + 1) * 128], A1_s[:, kt],
                            start=(kt == 0), stop=(kt == 3))
                    nc.scalar.activation(ATb[:, mt], ps, AF.Relu,
                                         bias=b1b_s[:, mt:mt + 1])
                    nc.scalar.activation(AT_s[:, mt], ps, AF.Relu,
                                         bias=b1b_s[:, mt:mt + 1])
                # P = A@W1'.T
                ps = epp.tile([32, NS], F32, tag="encp")
                for kt in range(2):
                    nc.tensor.matmul(ps, mw1_s[:, kt], ATb[:, kt],
                                     start=(kt == 0), stop=(kt == 1))
                nc.scalar.activation(PTb[:], ps, AF.Identity, bias=0.0)
                # P repacked for 2-chunk h1 passes (chunk 2g+j on partition
                # base 64j; PE matmul operands may only start at partition
                # 0/32/64): P4[64j+p, g, n] = P[p, (2g+j)*NCH+n]
                pv = PTb[:].rearrange("p (g j n) -> p g j n", g=4, j=2)
                for j in range(2):
                    nc.sync.dma_start(P4[64 * j:64 * j + 32, :, :], pv[:, :, j])
                # h1 stage 1 (DVE, overlaps the lab encoder): broadcast P
                for g in range(4):
                    nc.vector.tensor_copy(
                        h1g[g][:].rearrange("p (m n) -> p m n", n=NCH),
                        P4[:, g][:, None, :].broadcast_to((128, M, NCH)))

                # lab encoder (convs2 weights)
                for mt in range(4):
                    ps = epp.tile([128, M], F32, tag="encb")
                    for kt in range(3):
                        nc.tensor.matmul(
                            ps, w2a_s[:, kt, mt * 128:(mt + 1) * 128], lT_s[:, kt],
                            start=(kt == 0), stop=(kt == 2))
                    nc.scalar.activation(B1_s[:, mt], ps, AF.Relu,
                                         bias=b2a_s[:, mt:mt + 1])
                for mt in range(2):
                    ps = epp.tile([128, M], F32, tag="encb")
                    for kt in range(4):
                        nc.tensor.matmul(
                            ps, w2b_s[:, kt, mt * 128:(mt + 1) * 128], B1_s[:, kt],
                            start=(kt == 0), stop=(kt == 3))
                    nc.scalar.activation(BTb[:, mt], ps, AF.Relu,
                                         bias=b2b_s[:, mt:mt + 1])
                # R = mb1 - B@W1'.T
                ps = epp.tile([32, M], F32, tag="encp")
                for kt in range(2):
                    nc.tensor.matmul(ps, mw1_s[:, kt], BTb[:, kt],
                                     start=(kt == 0), stop=(kt == 1))
                nc.scalar.activation(RTb[:], ps, AF.Identity,
                                     bias=mb1_s[:], scale=-1.0)
                for j in range(4):
                    nc.sync.dma_start(R4[32 * j:32 * (j + 1), :], RTb[:])
                # R/B broadcast operands, materialized by flat doubling
                # copies (DVE fast mode)
                nc.vector.tensor_copy(R4_rep[:, :, 0:1], R4[:, :, None])
                w = 1
                while w < NCH:
                    nc.vector.tensor_copy(R4_rep[:, :, w:2 * w],
                                          R4_rep[:, :, 0:w])
                    w *= 2
                # h1 stage 2: += R, then relu (first two groups first so
                # chunk 0 unblocks ASAP)
                for g in range(2):
                    nc.vector.tensor_tensor(h1g[g][:].rearrange(
                        "p (m n) -> p m n", n=NCH), h1g[g][:].rearrange(
                        "p (m n) -> p m n", n=NCH), R4_rep[:], ALU.add)
                    nc.vector.tensor_scalar_max(h1g[g][:], h1g[g][:], 0.0)
                nc.vector.tensor_copy(B_rep[:, :, :, 0:1], BTb[:, :, :, None])
                w = 1
                while w < NCH:
                    nc.vector.tensor_copy(B_rep[:, :, :, w:2 * w],
                                          B_rep[:, :, :, 0:w])
                    w *= 2
                for g in range(2, 4):
                    nc.vector.tensor_tensor(h1g[g][:].rearrange(
                        "p (m n) -> p m n", n=NCH), h1g[g][:].rearrange(
                        "p (m n) -> p m n", n=NCH), R4_rep[:], ALU.add)
                    nc.vector.tensor_scalar_max(h1g[g][:], h1g[g][:], 0.0)

            # ---- hot loop over point chunks; pair cols ordered (m, n) ----
            # EEB planes: 0-1 = E (c-halves), 2-3 = E*B. One halving-tree
            # instruction per level covers both reductions (Z and U).
            # h1 is computed one GROUP (2 chunks, on partition bases 0/64)
            # at a time; later groups are emitted mid-loop so the in-order
            # DVE queue has work while the scalar engine finishes exp.
            ZU_s = wp.tile([128, 4, NS], F32)  # planes 0-1 Z, 2-3 U
            ATT = wp.tile([128, 2, NS], F32)

            with (
                tc.tile_pool(name="work", bufs=4) as hp,
                tc.tile_pool(name="big", bufs=3) as bp,
                tc.tile_pool(name="mm_psum", bufs=2, space="PSUM") as h2pp,
                tc.tile_pool(name="y3_psum", bufs=2, space="PSUM") as y3pp,
                tc.tile_pool(name="fc_psum", bufs=1, space="PSUM") as fpp,
            ):
                fps = fpp.tile([OUT_C, NS], F32)
                for c in range(NCHUNKS):
                    nsl = slice(c * NCH, (c + 1) * NCH)
                    h1 = h1g[c // 2][64 * (c % 2):64 * (c % 2) + 32, :]
                    EEB = bp.tile([128, 4, M * NCH], BF, tag="EEB")
                    for t in range(4):  # 1024-col groups (32 m-values each)
                        h2s = hp.tile([64, 1024], BF, tag="h2")
                        for bb in range(2):
                            ps2 = h2pp.tile([64, 512], F32, tag="h2ps")
                            q = 64 * (c % 2)
                            nc.tensor.matmul(
                                ps2, mw2b[q:q + 32, :],
                                h1[:, t * 1024 + bb * 512:
                                     t * 1024 + (bb + 1) * 512],
                                start=True, stop=True)
                            nc.scalar.activation(h2s[:, bb * 512:(bb + 1) * 512],
                                                 ps2, AF.Relu, bias=mb2_s[:])
                        for h in range(2):
                            ps3 = y3pp.tile([128, 1024], F32, tag="y3ps")
                            for bb in range(2):
                                nc.tensor.matmul(
                                    ps3[:, bb * 512:(bb + 1) * 512],
                                    mw3b[:, h * 128:(h + 1) * 128],
                                    h2s[:, bb * 512:(bb + 1) * 512],
                                    start=True, stop=True)
                            nc.scalar.activation(
                                EEB[:, h, t * 1024:(t + 1) * 1024], ps3, AF.Exp,
                                bias=mb3_s[:, h:h + 1])

                    # E = max(E', 1);  EB = E * B[m]   (flat)
                    nc.vector.tensor_scalar_max(EEB[:, 0:2], EEB[:, 0:2], 1.0)
                    nc.vector.tensor_tensor(EEB[:, 2:4], EEB[:, 0:2], B_rep[:],
                                            ALU.mult)
                    # Z/U: in-place halving tree over m (flat slices), one
                    # instruction per level across all 4 planes
                    v = EEB[:].rearrange("p q (m n) -> p q m n", n=NCH)
                    L = M // 2
                    while L >= 2:
                        nc.vector.tensor_tensor(
                            v[:, :, 0:L], v[:, :, 0:L], v[:, :, L:2 * L],
                            ALU.add)
                        L //= 2
                    nc.vector.tensor_tensor(
                        ZU_s[:, :, nsl], v[:, :, 0], v[:, :, 1], ALU.add)

                    # group tail: att = A - U/Z and its fc contribution for
                    # this group's points, overlapped with later chunks
                    if c % 2 == 1:
                        gsl = slice((c - 1) * NCH, (c + 1) * NCH)
                        Zr = hp.tile([128, 2, 2 * NCH], F32, tag="zr")
                        nc.vector.reciprocal_approx_fast(Zr[:],
                                                         ZU_s[:, 0:2, gsl])
                        nc.vector.tensor_tensor(Zr[:], ZU_s[:, 2:4, gsl],
                                                Zr[:], ALU.mult)
                        nc.vector.tensor_tensor(ATT[:, :, gsl],
                                                AT_s[:, :, gsl], Zr[:],
                                                ALU.subtract)
                        for kt in range(2):
                            nc.tensor.matmul(fps[:, gsl], fcw_s[:, kt],
                                             ATT[:, kt, gsl],
                                             start=(kt == 0), stop=(kt == 1))

                outT = wp.tile([OUT_C, NS], F32)
                nc.scalar.activation(outT[:], fps, AF.Identity, bias=fcb_s[:])
                nc.sync.dma_start(out_d[:], outT[:])

    nc.finalize()
    return nc


def _fold(w, b, g, be):
    w = np.asarray(w, np.float32)
    b = np.asarray(b, np.float32)
    g = np.asarray(g, np.float32)
    be = np.asarray(be, np.float32)
    return (g[:, None] * w).astype(np.float32), (g * b + be).astype(np.float32)


def _padk(wT, k_to):  # pad contraction (row) dim with zeros
    out = np.zeros((k_to, wT.shape[1]), np.float32)
    out[: wT.shape[0]] = wT
    return out


def _pack_block(buf, spec, offs, name, arr, rows=128):
    """arr: [rows, w] block -> buf[:rows, off:off+w]."""
    off, w = offs[name], dict(spec)[name]
    assert arr.shape == (rows, w), (name, arr.shape, rows, w)
    buf[:rows, off:off + w] = arr


def _kt(wT):  # [K, m] -> [128, K/128 * m] partition-tiled layout
    k, m = wT.shape
    return wT.reshape(k // 128, 128, m).transpose(1, 0, 2).reshape(128, -1)


def kernel(**inputs):
    if "prog" not in _PROG_CACHE:
        _PROG_CACHE["prog"] = _build_program()
    nc = _PROG_CACHE["prog"]

    f = {k: np.asarray(v, np.float32) for k, v in inputs.items()}
    w1a, b1a = _fold(f["w1a"], f["b1a"], f["g1a"], f["be1a"])
    w1b, b1b = _fold(f["w1b"], f["b1b"], f["g1b"], f["be1b"])
    w2a, b2a = _fold(f["w2a"], f["b2a"], f["g2a"], f["be2a"])
    w2b, b2b = _fold(f["w2b"], f["b2b"], f["g2b"], f["be2b"])
    mw1, mb1 = _fold(f["mw1"], f["mb1"], f["mg1"], f["mbe1"])
    mw2, mb2 = _fold(f["mw2"], f["mb2"], f["mg2"], f["mbe2"])
    mw3, mb3 = _fold(f["mw3"], f["mb3"], f["mg3"], f["mbe3"])

    base = np.zeros((128, PACKF), np.float32)

    def pb(name, arr, rows=128):
        _pack_block(base, _PACK_SPEC, _PACK_OFF, name, arr, rows)

    pb("fcw", _kt(f["fcw"].T))
    pb("b1a", b1a.reshape(4, 128).T)
    pb("b1b", b1b.reshape(2, 128).T)
    pb("b2a", b2a.reshape(4, 128).T)
    pb("b2b", b2b.reshape(2, 128).T)
    pb("mb1", mb1.reshape(32, 1), rows=32)
    pb("mb2", mb2.reshape(64, 1), rows=64)
    pb("mb3", mb3.reshape(2, 128).T)
    pb("fcb", f["fcb"].reshape(OUT_C, 1), rows=OUT_C)

    baseb = np.zeros((128, PACKBF), np.float32)

    def pbb(name, arr, rows=128):
        _pack_block(baseb, _PACKB_SPEC, _PACKB_OFF, name, arr, rows)

    pbb("w1a", _kt(_padk(w1a.T, KIN)))
    pbb("w1b", _kt(w1b.T))
    pbb("w2a", _kt(_padk(w2a.T, KIN)))
    pbb("w2b", _kt(w2b.T))
    pbb("mw1", _kt(mw1.T))
    pbb("mw2", np.tile(mw2.T, (4, 1)))
    pbb("mw3", mw3.T, rows=64)
    pbb("lT", _kt(_padk(f["lab_fea"].T, KIN)))

    in_maps = []
    for i in range(NCORES):
        buf = baseb.copy()
        shard = f["ext_fea"][i * NS:(i + 1) * NS]
        _pack_block(buf, _PACKB_SPEC, _PACKB_OFF, "xT",
                    _kt(_padk(shard.T, KIN)))
        in_maps.append({
            "pack": np.ascontiguousarray(base),
            "packb": np.ascontiguousarray(buf.astype(BF_NP)),
        })

    res = run_bass_kernel_spmd(nc, in_maps, core_ids=list(range(NCORES)),
                               tmpdir=os.environ.get("KERNEL_TRACE_DIR"))
    _PROG_CACHE["last_res"] = res
    return np.concatenate(
        [np.ascontiguousarray(res.results[i]["out"].T) for i in range(NCORES)],
        axis=0)


if __name__ == "__main__":
    pass
